# revision 6
# baseline (speedup 1.0000x reference)
"""Multi-head causal attention (B=4, T=2048, 16 heads x 64) on 8 trn2 NeuronCores.

Sharding: tensor-parallel over heads (2 heads/core) for QKV projection +
attention; AllToAll reshard (head-sharded -> token-sharded) before the output
projection; each core computes the output projection for its 1024 tokens.

Per-core dataflow (all matmuls in fp32r = full-rate fp32 on the PE):
  - x.T (host-pretransposed) streamed per (batch, 512-token tile); QKV
    projections produce Q.T, K.T [128=2*64 headdim, T] and V.T; V.T is
    PE-transposed into V [ks, d] with an appended ones-column (V_aug).
  - Scores are computed transposed, S.T[ks,tq] = K @ Q.T, two heads packed
    into the 128-row PE array via tile_position row groups (K=64 each).
  - softmax without max-subtraction (scores provably in [-0.52, 0.52]):
    E = exp(S.T * 0.125) on ACT straight out of PSUM; causal masking via
    skipping fully-masked tiles, sub-column exp on diagonal tiles and one
    128x128 triangular mask multiply.
  - A'V and the softmax denominator in one accumulation: lhsT = V_aug
    [ks, 65] (col 64 = ones) -> O.T|denom [65, tq] accumulated over ks.
  - normalize: reciprocal of denom row, gpsimd partition-broadcast, DVE mul.
  - AllToAll of O.T blocks [8, 128, 1024]; output projection over the 8
    gathered head-chunks; bias add; each core writes tokens
    [rank*1024, (rank+1)*1024).
"""

import numpy as np

import concourse.bass as bass
import concourse.bacc as bacc
import concourse.tile as tile
from concourse import mybir
from concourse.bass_utils import run_bass_kernel_spmd

NCORES = 8
B, T, C, H, D = 4, 2048, 1024, 16, 64
TQ = 512          # moving-dim tile for scores / A'V
NKC = T // 128    # ks 128-chunks per batch (16)
NJ = T // TQ      # tq tiles per batch (4)
NCC = C // 128    # contraction chunks for projections (8)

f32 = mybir.dt.float32
f32r = mybir.dt.float32r
AF = mybir.ActivationFunctionType


def build_bass():
    nc = bacc.Bacc(None, num_devices=NCORES)

    xT = nc.dram_tensor("xT", [B, C, T], f32, kind="ExternalInput")
    # per-core qkv weights: [3(q,k,v), chunk, row-in-chunk, 2*D]
    w_in = nc.dram_tensor("w", [3, NCC, 128, 2 * D], f32, kind="ExternalInput")
    # proj_w.T chunked: [chunk, row-in-chunk, C]
    pw_in = nc.dram_tensor("pw", [NCC, 128, C], f32, kind="ExternalInput")
    bias_in = nc.dram_tensor("biasb", [128, C], f32, kind="ExternalInput")
    y_out = nc.dram_tensor("y", [B * T // NCORES, C], f32, kind="ExternalOutput")

    ident_np = np.eye(128, dtype=np.float32)
    tri_np = (np.arange(128)[None, :] >= np.arange(128)[:, None]).astype(np.float32)
    ident_dram = nc.inline_tensor(ident_np, name="ident")
    ones_dram = nc.inline_tensor(np.ones((128, 2), dtype=np.float32), name="onescol")
    tri_dram = nc.inline_tensor(tri_np, name="trimask")

    with tile.TileContext(nc, num_cores=NCORES) as tc:
        with tc.tile_pool(name="dram", bufs=1, space="DRAM") as dpool:
            a2a_send = dpool.tile([NCORES, 2 * D, 1024], f32)
            a2a_recv = dpool.tile([NCORES, 2 * D, 1024], f32)

            with (
                tc.tile_pool(name="consts", bufs=1) as consts,
                tc.tile_pool(name="xt", bufs=3) as xt_pool,
                tc.tile_pool(name="qt", bufs=2) as qt_pool,
                tc.tile_pool(name="kt", bufs=2) as kt_pool,
                tc.tile_pool(name="vt", bufs=2) as vt_pool,
                tc.tile_pool(name="vaug", bufs=2) as vaug_pool,
                tc.tile_pool(name="e", bufs=4) as e_pool,
                tc.tile_pool(name="onorm", bufs=4) as onorm_pool,
                tc.tile_pool(name="small", bufs=4) as small_pool,
                tc.tile_pool(name="ps_a", bufs=2, space="PSUM") as ps_a,
                tc.tile_pool(name="ps_s", bufs=3, space="PSUM") as ps_s,
                tc.tile_pool(name="ps_o", bufs=1, space="PSUM") as ps_o,
            ):
                w_sb = consts.tile([128, 3, NCC, 2 * D], f32r)
                nc.sync.dma_start(out=w_sb[:], in_=w_in.rearrange("p i r c -> r p i c").bitcast(f32r))
                ident_sb = consts.tile([128, 128], f32)
                ones_sb = consts.tile([128, 2], f32r)
                nc.sync.dma_start(out=ones_sb[:], in_=ones_dram[:].bitcast(f32r))
                nc.sync.dma_start(out=ident_sb[:], in_=ident_dram[:])
                tri_sb = consts.tile([128, 128], f32r)
                nc.sync.dma_start(out=tri_sb[:], in_=tri_dram[:].bitcast(f32r))

                for b in range(B):
                    QT = qt_pool.tile([128, T], f32r)
                    KT = kt_pool.tile([128, T], f32r)
                    VT = vt_pool.tile([128, T], f32)
                    dests = [QT, KT, VT]
                    for t4 in range(NJ):
                        xt = xt_pool.tile([128, NCC, TQ], f32r)
                        nc.sync.dma_start(
                            out=xt[:],
                            in_=xT[b, :, t4 * TQ:(t4 + 1) * TQ].rearrange(
                                "(i p) t -> p i t", p=128
                            ).bitcast(f32r),
                        )
                        for p3 in range(3):
                            ps = ps_a.tile([128, TQ], f32, tag="qkv")
                            for i in range(NCC):
                                nc.tensor.matmul(
                                    ps[:],
                                    lhsT=w_sb[:, p3, i, :],
                                    rhs=xt[:, i, :],
                                    start=(i == 0),
                                    stop=(i == NCC - 1),
                                )
                            nc.vector.tensor_copy(
                                dests[p3][:, t4 * TQ:(t4 + 1) * TQ], ps[:]
                            )

                    # V.T -> V_aug [ks-chunk, (h d | ones)] via PE transpose
                    VA = vaug_pool.tile([128, NKC, 130], f32r)
                    for kc in range(NKC):
                        pst = ps_s.tile([128, 128], f32, tag="s", name=f"pst_{b}_{kc}")
                        nc.tensor.transpose(
                            pst[:], VT[:, kc * 128:(kc + 1) * 128], ident_sb[:]
                        )
                        # cols 0..63 -> VA[..,0:64], cols 64..127 -> VA[..,65:129]
                        out_ap = VA[:, kc, :].rearrange("p (g s) -> p g s", s=65)[
                            :, :, 0:64
                        ]
                        in_ap = pst[:].rearrange("p (g s) -> p g s", s=64)
                        nc.vector.tensor_copy(out_ap, in_ap)
                        ones_ap = VA[:, kc, :].rearrange("p (g s) -> p g s", s=65)[
                            :, :, 64
                        ]
                        nc.vector.tensor_copy(ones_ap, ones_sb[:])

                    # attention for this batch, both heads
                    for j in range(NJ):
                        po = [
                            ps_o.tile([65, TQ], f32, tag=f"o{h}", name=f"po{h}_{b}_{j}")
                            for h in (0, 1)
                        ]
                        nchunks = 4 * (j + 1)
                        for c in range(nchunks):
                            m = c - 4 * j  # >=0 on diagonal tiles
                            for h in (0, 1):
                                pss = ps_s.tile([128, TQ], f32, tag="s", name=f"pss_{b}_{j}_{c}_{h}")
                                nc.tensor.matmul(
                                    pss[:],
                                    lhsT=KT[
                                        64 * h:64 * (h + 1), c * 128:(c + 1) * 128
                                    ],
                                    rhs=QT[
                                        64 * h:64 * (h + 1), j * TQ:(j + 1) * TQ
                                    ],
                                    start=True,
                                    stop=True,
                                    tile_position=(64 * h, 0),
                                )
                                E = e_pool.tile([128, TQ], f32r)
                                if m < 0:
                                    nc.scalar.activation(
                                        E[:], pss[:], AF.Exp, scale=0.125
                                    )
                                    nc.tensor.matmul(
                                        po[h][:, :],
                                        lhsT=VA[:, c, 65 * h:65 * h + 65],
                                        rhs=E[:],
                                        start=(c == 0),
                                        stop=(c == nchunks - 1),
                                    )
                                else:
                                    cs = slice(m * 128, TQ)
                                    nc.scalar.activation(
                                        E[:, cs], pss[:, cs], AF.Exp, scale=0.125
                                    )
                                    nc.vector.tensor_mul(
                                        E[:, m * 128:(m + 1) * 128],
                                        E[:, m * 128:(m + 1) * 128],
                                        tri_sb[:],
                                    )
                                    nc.tensor.matmul(
                                        po[h][:, cs],
                                        lhsT=VA[:, c, 65 * h:65 * h + 65],
                                        rhs=E[:, cs],
                                        start=(c == 0),
                                        stop=(c == nchunks - 1),
                                    )
                        # normalize + ship to A2A send buffer
                        blk = 2 * b + j // 2
                        j2 = j % 2
                        for h in (0, 1):
                            rec = small_pool.tile([1, TQ], f32, tag="rec")
                            nc.vector.reciprocal(rec[:], po[h][64:65, :])
                            den = small_pool.tile([64, TQ], f32, tag="den")
                            nc.gpsimd.partition_broadcast(den[:], rec[:])
                            on = onorm_pool.tile([64, TQ], f32)
                            nc.vector.tensor_mul(on[:], po[h][0:64, :], den[:])
                            nc.sync.dma_start(
                                out=a2a_send[
                                    blk, 64 * h:64 * (h + 1), j2 * TQ:(j2 + 1) * TQ
                                ],
                                in_=on[:],
                            )

            nc.gpsimd.collective_compute(
                "AllToAll",
                mybir.AluOpType.bypass,
                replica_groups=[list(range(NCORES))],
                ins=[a2a_send[:].opt()],
                outs=[a2a_recv[:].opt()],
            )

            # output projection on this core's 1024 tokens
            with (
                tc.tile_pool(name="proj", bufs=1) as proj_pool,
                tc.tile_pool(name="ytile", bufs=4) as y_pool,
                tc.tile_pool(name="ps_p", bufs=4, space="PSUM") as ps_p,
            ):
                oall = proj_pool.tile([128, NCORES, 1024], f32r)
                nc.sync.dma_start(out=oall[:], in_=a2a_recv[:].rearrange("i p t -> p i t").bitcast(f32r))
                pw_sb = proj_pool.tile([128, NCC, C], f32r)
                nc.sync.dma_start(out=pw_sb[:], in_=pw_in.rearrange("i r e -> r i e").bitcast(f32r))
                bias_sb = proj_pool.tile([128, C], f32)
                nc.sync.dma_start(out=bias_sb[:], in_=bias_in[:])

                for s in range(8):          # token sub-tiles of 128
                    for n in range(2):      # out-embd halves of 512
                        pso = ps_p.tile([128, 512], f32)
                        for i in range(NCC):
                            nc.tensor.matmul(
                                pso[:],
                                lhsT=oall[:, i, s * 128:(s + 1) * 128],
                                rhs=pw_sb[:, i, n * 512:(n + 1) * 512],
                                start=(i == 0),
                                stop=(i == NCC - 1),
                            )
                        yt = y_pool.tile([128, 512], f32)
                        nc.vector.tensor_add(
                            yt[:], pso[:], bias_sb[:, n * 512:(n + 1) * 512]
                        )
                        nc.sync.dma_start(
                            out=y_out[s * 128:(s + 1) * 128, n * 512:(n + 1) * 512],
                            in_=yt[:],
                        )
    nc.finalize()
    return nc


_NC_CACHE = {}


def _get_nc():
    if "nc" not in _NC_CACHE:
        _NC_CACHE["nc"] = build_bass()
    return _NC_CACHE["nc"]


def _prep_inputs(x, Wk, Wq, Wv, proj_w, proj_b):
    x = np.ascontiguousarray(np.asarray(x, dtype=np.float32))
    xT = np.ascontiguousarray(x.transpose(0, 2, 1))  # [B, C, T]
    pw_r = np.ascontiguousarray(np.asarray(proj_w, np.float32).T).reshape(NCC, 128, C)
    biasb = np.ascontiguousarray(
        np.broadcast_to(np.asarray(proj_b, np.float32), (128, C))
    )
    in_maps = []
    for core in range(NCORES):
        h0 = 2 * core

        def pack(W):
            W2 = np.concatenate(
                [np.asarray(W[h0], np.float32), np.asarray(W[h0 + 1], np.float32)],
                axis=1,
            )  # [C, 2D]
            return W2.reshape(NCC, 128, 2 * D)

        wq = np.stack([pack(Wq), pack(Wk), pack(Wv)], axis=0)  # [3, NCC, 128, 2D]
        in_maps.append(
            {
                "xT": xT,
                "w": np.ascontiguousarray(wq),
                "pw": pw_r,
                "biasb": biasb,
            }
        )
    return in_maps


def kernel(x, Wk, Wq, Wv, proj_w, proj_b, _trace=False, _trace_kwargs=None):
    in_maps = _prep_inputs(x, Wk, Wq, Wv, proj_w, proj_b)
    nc = _get_nc()
    kw = {}
    if _trace:
        kw = dict(trace=True, trace_kwargs=_trace_kwargs or {})
    res = run_bass_kernel_spmd(nc, in_maps, core_ids=list(range(NCORES)), **kw)
    y = np.concatenate([res.results[c]["y"] for c in range(NCORES)], axis=0)
    out = y.reshape(B, T, C)
    if _trace:
        return out, res
    return out


if __name__ == "__main__":
    d = np.load("/root/problem/cache_io.npz")
    out = kernel(d["x"], d["Wk"], d["Wq"], d["Wv"], d["proj_w"], d["proj_b"])
    ref = d["ref"]
    err = np.abs(out - ref).max() / np.abs(ref).max()
    print("Relative error:", err)


# revision 9
# speedup vs baseline: 1.2021x; 1.2021x over previous
"""Multi-head causal attention (B=4, T=2048, 16 heads x 64) on 8 trn2 NeuronCores.

Sharding: tensor-parallel over heads (2 heads/core) for QKV projection +
attention; two AllToAll reshards (head-sharded -> token-sharded), the first
fired mid-kernel so it overlaps batches 2-3; each core then computes the
output projection for its 1024 tokens (two 512-token shards).

Per-core dataflow (all matmuls in fp32r = full-rate fp32 on the PE):
  - x.T (host-pretransposed) streamed per (batch, 512-token tile); QKV
    projections produce Q.T, K.T [128=2*64 headdim, T] and V.T; V.T is
    PE-transposed into V [ks, d] with an appended ones-column (V_aug).
  - Scores are computed transposed, S.T[ks,tq] = K @ Q.T (K=64 contraction),
    two ks-chunks paired into one [128,1024] PSUM tile so the Exp activation
    runs once per pair; softmax without max-subtraction (scores provably in
    [-0.52, 0.52]); causal masking via skipping fully-masked tiles, one
    128x128 triangular mask multiply per diagonal chunk.
  - A'V and the softmax denominator in one accumulation: lhsT = V_aug
    [ks, 65] (col 64 = ones) -> O.T|denom [65, tq] accumulated over ks.
    The score matmuls for pair k+1 are emitted before the A'V matmuls of
    pair k so the PE never stalls waiting for the Exp.
  - normalize: reciprocal of denom row, gpsimd partition-broadcast, DVE mul.
  - output projection over the 8 gathered head-chunks; bias add.
"""

import numpy as np

import concourse.bacc as bacc
import concourse.tile as tile
from concourse import mybir
from concourse.bass_utils import run_bass_kernel_spmd

NCORES = 8
B, T, C, H, D = 4, 2048, 1024, 16, 64
TQ = 512          # moving-dim tile for scores / A'V
NKC = T // 128    # ks 128-chunks per batch (16)
NJ = T // TQ      # tq tiles per batch (4)
NCC = C // 128    # contraction chunks for projections (8)

f32 = mybir.dt.float32
f32r = mybir.dt.float32r
AF = mybir.ActivationFunctionType


def build_bass():
    nc = bacc.Bacc(None, num_devices=NCORES)

    xT = nc.dram_tensor("xT", [B, C, T], f32, kind="ExternalInput")
    # per-core qkv weights: [3(q,k,v), chunk, row-in-chunk, 2*D]
    w_in = nc.dram_tensor("w", [3, NCC, 128, 2 * D], f32, kind="ExternalInput")
    # proj_w.T chunked: [chunk, row-in-chunk, C]
    pw_in = nc.dram_tensor("pw", [NCC, 128, C], f32, kind="ExternalInput")
    bias_in = nc.dram_tensor("biasb", [128, C], f32, kind="ExternalInput")
    y_out = nc.dram_tensor("y", [B * T // NCORES, C], f32, kind="ExternalOutput")

    ident_np = np.eye(128, dtype=np.float32)
    tri_np = (np.arange(128)[None, :] >= np.arange(128)[:, None]).astype(np.float32)
    ident_dram = nc.inline_tensor(ident_np, name="ident")
    tri_dram = nc.inline_tensor(tri_np, name="trimask")
    onescols_dram = nc.inline_tensor(np.ones((128, NKC), np.float32), name="onescols")

    with tile.TileContext(nc, num_cores=NCORES) as tc:
        with tc.tile_pool(name="dram", bufs=1, space="DRAM") as dpool:
            # one [128hd, 512tok] shard per (b, j); first A2A covers b=0,1
            send1 = dpool.tile([NCORES, 2 * D, TQ], f32)
            recv1 = dpool.tile([NCORES, 2 * D, TQ], f32)
            send2 = dpool.tile([NCORES, 2 * D, TQ], f32)
            recv2 = dpool.tile([NCORES, 2 * D, TQ], f32)

            def fire_a2a(send, recv):
                nc.gpsimd.collective_compute(
                    "AllToAll",
                    mybir.AluOpType.bypass,
                    replica_groups=[list(range(NCORES))],
                    ins=[send[:].opt()],
                    outs=[recv[:].opt()],
                )

            with (
                tc.tile_pool(name="consts", bufs=1) as consts,
                tc.tile_pool(name="xt", bufs=4) as xt_pool,
                tc.tile_pool(name="qt", bufs=2) as qt_pool,
                tc.tile_pool(name="kt", bufs=2) as kt_pool,
                tc.tile_pool(name="vt", bufs=2) as vt_pool,
                tc.tile_pool(name="vaug", bufs=2) as vaug_pool,
                tc.tile_pool(name="e", bufs=3) as e_pool,
                tc.tile_pool(name="onorm", bufs=4) as onorm_pool,
                tc.tile_pool(name="small", bufs=4) as small_pool,
                tc.tile_pool(name="ps_a", bufs=2, space="PSUM") as ps_a,
                tc.tile_pool(name="ps_s", bufs=2, space="PSUM") as ps_s,
                tc.tile_pool(name="ps_o", bufs=1, space="PSUM") as ps_o,
            ):
                w_sb = consts.tile([128, 3, NCC, 2 * D], f32r)
                nc.sync.dma_start(
                    out=w_sb[:], in_=w_in.rearrange("p i r c -> r p i c").bitcast(f32r)
                )
                ident_sb = consts.tile([128, 128], f32)
                nc.sync.dma_start(out=ident_sb[:], in_=ident_dram[:])
                tri_sb = consts.tile([128, 128], f32r)
                nc.sync.dma_start(out=tri_sb[:], in_=tri_dram[:].bitcast(f32r))

                for b in range(B):
                    QT = qt_pool.tile([128, T], f32r)
                    KT = kt_pool.tile([128, T], f32r)
                    VT = vt_pool.tile([128, T], f32)
                    dests = [QT, KT, VT]
                    for t4 in range(NJ):
                        xt = xt_pool.tile([128, NCC, TQ], f32r)
                        nc.sync.dma_start(
                            out=xt[:],
                            in_=xT[b, :, t4 * TQ:(t4 + 1) * TQ]
                            .rearrange("(i p) t -> p i t", p=128)
                            .bitcast(f32r),
                        )
                        for p3 in range(3):
                            ps = ps_a.tile([128, TQ], f32, tag="qkv")
                            for i in range(NCC):
                                nc.tensor.matmul(
                                    ps[:],
                                    lhsT=w_sb[:, p3, i, :],
                                    rhs=xt[:, i, :],
                                    start=(i == 0),
                                    stop=(i == NCC - 1),
                                )
                            nc.vector.tensor_copy(
                                dests[p3][:, t4 * TQ:(t4 + 1) * TQ], ps[:]
                            )

                    # V.T -> V_aug [ks-chunk, (h0 d | ones | h1 d | ones)]
                    VA = vaug_pool.tile([128, NKC, 130], f32r)
                    nc.sync.dma_start(
                        out=VA[:, :, 64], in_=onescols_dram[:].bitcast(f32r)
                    )
                    nc.sync.dma_start(
                        out=VA[:, :, 129], in_=onescols_dram[:].bitcast(f32r)
                    )
                    for kc in range(NKC):
                        pst = ps_a.tile([128, 128], f32, tag="qkv", name=f"pst{b}_{kc}")
                        nc.tensor.transpose(
                            pst[:], VT[:, kc * 128:(kc + 1) * 128], ident_sb[:]
                        )
                        out_ap = VA[:, kc, :].rearrange("p (g s) -> p g s", s=65)[
                            :, :, 0:64
                        ]
                        in_ap = pst[:].rearrange("p (g s) -> p g s", s=64)
                        nc.vector.tensor_copy(out_ap, in_ap)

                    # attention for this batch, both heads, chunk-PAIR pipelined
                    for j in range(NJ):
                        po = [
                            ps_o.tile([65, TQ], f32, tag=f"o{h}", name=f"po{h}_{b}_{j}")
                            for h in (0, 1)
                        ]
                        npairs = 2 * (j + 1)
                        av_queue = []  # exp'd pairs whose A'V is pending

                        def emit_av(item, j=j, po=po):
                            E2, cpair = item
                            for h in (0, 1):
                                for ci in (0, 1):
                                    c = 2 * cpair + ci
                                    m = c - 4 * j
                                    cs = (
                                        slice(ci * TQ, (ci + 1) * TQ)
                                        if m < 0
                                        else slice(ci * TQ + m * 128, (ci + 1) * TQ)
                                    )
                                    ocs = (
                                        slice(0, TQ) if m < 0 else slice(m * 128, TQ)
                                    )
                                    nc.tensor.matmul(
                                        po[h][:, ocs],
                                        lhsT=VA[:, c, 65 * h:65 * h + 65],
                                        rhs=E2[h][:, cs],
                                        start=(c == 0),
                                        stop=(c == 4 * j + 3),
                                    )

                        for cpair in range(npairs):
                            E2 = []
                            for h in (0, 1):
                                pss = ps_s.tile(
                                    [128, 2 * TQ], f32, tag="pss", name=f"pss{b}_{j}_{cpair}_{h}"
                                )
                                for ci in (0, 1):
                                    c = 2 * cpair + ci
                                    nc.tensor.matmul(
                                        pss[:, ci * TQ:(ci + 1) * TQ],
                                        lhsT=KT[
                                            64 * h:64 * (h + 1),
                                            c * 128:(c + 1) * 128,
                                        ],
                                        rhs=QT[
                                            64 * h:64 * (h + 1), j * TQ:(j + 1) * TQ
                                        ],
                                        start=True,
                                        stop=True,
                                        tile_position=(64 * h, 0),
                                    )
                                E = e_pool.tile(
                                    [128, 2 * TQ], f32r, tag="E", name=f"E{b}_{j}_{cpair}_{h}"
                                )
                                nc.scalar.activation(E[:], pss[:], AF.Exp, scale=0.125)
                                # triangular mask on diagonal chunks
                                for ci in (0, 1):
                                    c = 2 * cpair + ci
                                    m = c - 4 * j
                                    if m >= 0:
                                        sl = slice(
                                            ci * TQ + m * 128, ci * TQ + (m + 1) * 128
                                        )
                                        nc.vector.tensor_mul(
                                            E[:, sl], E[:, sl], tri_sb[:]
                                        )
                                E2.append(E)
                            av_queue.append((E2, cpair))
                            if len(av_queue) > 1:
                                emit_av(av_queue.pop(0))
                        emit_av(av_queue.pop(0))

                        # normalize + ship to A2A send buffer (shard = 4(b%2)+j)
                        send = send1 if b < 2 else send2
                        blk = 4 * (b % 2) + j
                        for h in (0, 1):
                            rec = small_pool.tile([1, TQ], f32, tag="rec")
                            nc.vector.reciprocal(rec[:], po[h][64:65, :])
                            den = small_pool.tile([64, TQ], f32, tag="den")
                            nc.gpsimd.partition_broadcast(den[:], rec[:])
                            on = onorm_pool.tile([64, TQ], f32)
                            nc.vector.tensor_mul(on[:], po[h][0:64, :], den[:])
                            nc.sync.dma_start(
                                out=send[blk, 64 * h:64 * (h + 1), :], in_=on[:]
                            )
                    if b == 1:
                        fire_a2a(send1, recv1)
                fire_a2a(send2, recv2)

            # output projection: rows 0:512 <- recv1 shard, rows 512:1024 <- recv2
            with (
                tc.tile_pool(name="proj", bufs=1) as proj_pool,
                tc.tile_pool(name="ytile", bufs=4) as y_pool,
                tc.tile_pool(name="ps_p", bufs=4, space="PSUM") as ps_p,
            ):
                pw_sb = proj_pool.tile([128, NCC, C], f32r)
                nc.sync.dma_start(
                    out=pw_sb[:], in_=pw_in.rearrange("i r e -> r i e").bitcast(f32r)
                )
                bias_sb = proj_pool.tile([128, C], f32)
                nc.sync.dma_start(out=bias_sb[:], in_=bias_in[:])
                for half, recv in ((0, recv1), (1, recv2)):
                    oall = proj_pool.tile([128, NCORES, TQ], f32r, tag="oall", bufs=2, name=f"oall{half}")
                    nc.sync.dma_start(
                        out=oall[:],
                        in_=recv[:].rearrange("i p t -> p i t").bitcast(f32r),
                    )
                    for s in range(4):          # token sub-tiles of 128
                        for n in range(2):      # out-embd halves of 512
                            pso = ps_p.tile([128, 512], f32, tag="pso", name=f"pso{half}_{s}_{n}")
                            for i in range(NCC):
                                nc.tensor.matmul(
                                    pso[:],
                                    lhsT=oall[:, i, s * 128:(s + 1) * 128],
                                    rhs=pw_sb[:, i, n * 512:(n + 1) * 512],
                                    start=(i == 0),
                                    stop=(i == NCC - 1),
                                )
                            yt = y_pool.tile([128, 512], f32)
                            nc.vector.tensor_add(
                                yt[:], pso[:], bias_sb[:, n * 512:(n + 1) * 512]
                            )
                            nc.sync.dma_start(
                                out=y_out[
                                    half * 512 + s * 128:half * 512 + (s + 1) * 128,
                                    n * 512:(n + 1) * 512,
                                ],
                                in_=yt[:],
                            )
    nc.finalize()
    return nc


_NC_CACHE = {}


def _get_nc():
    if "nc" not in _NC_CACHE:
        _NC_CACHE["nc"] = build_bass()
    return _NC_CACHE["nc"]


def _prep_inputs(x, Wk, Wq, Wv, proj_w, proj_b):
    x = np.ascontiguousarray(np.asarray(x, dtype=np.float32))
    xT = np.ascontiguousarray(x.transpose(0, 2, 1))  # [B, C, T]
    pw_r = np.ascontiguousarray(np.asarray(proj_w, np.float32).T).reshape(NCC, 128, C)
    biasb = np.ascontiguousarray(
        np.broadcast_to(np.asarray(proj_b, np.float32), (128, C))
    )
    in_maps = []
    for core in range(NCORES):
        h0 = 2 * core

        def pack(W):
            W2 = np.concatenate(
                [np.asarray(W[h0], np.float32), np.asarray(W[h0 + 1], np.float32)],
                axis=1,
            )  # [C, 2D]
            return W2.reshape(NCC, 128, 2 * D)

        wq = np.stack([pack(Wq), pack(Wk), pack(Wv)], axis=0)  # [3, NCC, 128, 2D]
        in_maps.append(
            {
                "xT": xT,
                "w": np.ascontiguousarray(wq),
                "pw": pw_r,
                "biasb": biasb,
            }
        )
    return in_maps


def _assemble(results):
    """Core r's y rows 0:512 = tokens of (b=r//4, j=r%4); rows 512:1024 =
    (b=2+r//4, j=r%4), where token block (b, j) = flat[b*2048+j*512 : +512]."""
    out = np.empty((B * T, C), np.float32)
    for r in range(NCORES):
        y = results[r]["y"]
        b1, j1 = r // 4, r % 4
        out[b1 * T + j1 * TQ:b1 * T + (j1 + 1) * TQ] = y[0:TQ]
        b2 = 2 + r // 4
        out[b2 * T + j1 * TQ:b2 * T + (j1 + 1) * TQ] = y[TQ:2 * TQ]
    return out.reshape(B, T, C)


def kernel(x, Wk, Wq, Wv, proj_w, proj_b, _trace=False, _trace_kwargs=None):
    in_maps = _prep_inputs(x, Wk, Wq, Wv, proj_w, proj_b)
    nc = _get_nc()
    kw = {}
    if _trace:
        kw = dict(trace=True, trace_kwargs=_trace_kwargs or {})
    res = run_bass_kernel_spmd(nc, in_maps, core_ids=list(range(NCORES)), **kw)
    out = _assemble(res.results)
    if _trace:
        return out, res
    return out


if __name__ == "__main__":
    d = np.load("/root/problem/cache_io.npz")
    out = kernel(d["x"], d["Wk"], d["Wq"], d["Wv"], d["proj_w"], d["proj_b"])
    ref = d["ref"]
    err = np.abs(out - ref).max() / np.abs(ref).max()
    print("Relative error:", err)


# revision 11
# speedup vs baseline: 1.3032x; 1.0842x over previous
"""Multi-head causal attention (B=4, T=2048, 16 heads x 64) on 8 trn2 NeuronCores.

Sharding: tensor-parallel over heads (2 heads/core) for QKV projection +
attention; one AllToAll reshard per batch (head-sharded -> token-sharded),
fired as each batch finishes so only the last one is exposed; the output
projection for each batch's tokens is interleaved one batch behind.

Per-core dataflow (all matmuls in fp32r = full-rate fp32 on the PE):
  - x.T (host-pretransposed) streamed per (batch, 512-token tile); QKV
    projections produce Q.T, K.T [128=2*64 headdim, T] and V.T; V.T is
    PE-transposed into V [ks, d] with an appended ones-column (V_aug).
  - Scores are computed transposed, S.T[ks,tq] = K @ Q.T (K=64 contraction),
    two ks-chunks paired into one [128,1024] PSUM tile so the Exp activation
    runs once per pair; softmax without max-subtraction (scores provably in
    [-0.52, 0.52]); causal masking via skipping fully-masked tiles, one
    128x128 triangular mask multiply per diagonal chunk.
  - A'V and the softmax denominator in one accumulation: lhsT = V_aug
    [ks, 65] (col 64 = ones) -> O.T|denom [65, tq] accumulated over ks.
    The score matmuls for pair k+1 are emitted before the A'V matmuls of
    pair k so the PE never stalls waiting for the Exp.
  - normalize: copy O.T out of PSUM first (frees the accumulator bank),
    then reciprocal of denom row, gpsimd partition-broadcast, DVE mul.
  - output projection over the 8 gathered head-chunks; bias add.

Each AllToAll moves [8 shards, 128 hd, 256 tok]: shard 2j+half of batch b
holds tokens b*2048 + j*512 + half*256 + [0,256); rank r therefore owns
tokens (j=r//2, half=r%2) of every batch = 4 x 256 rows of y.
"""

import numpy as np

import concourse.bacc as bacc
import concourse.tile as tile
from concourse import mybir
from concourse.bass_utils import run_bass_kernel_spmd

NCORES = 8
B, T, C, H, D = 4, 2048, 1024, 16, 64
TQ = 512          # moving-dim tile for scores / A'V
NKC = T // 128    # ks 128-chunks per batch (16)
NJ = T // TQ      # tq tiles per batch (4)
NCC = C // 128    # contraction chunks for projections (8)
TS = 256          # tokens per A2A shard

f32 = mybir.dt.float32
f32r = mybir.dt.float32r
AF = mybir.ActivationFunctionType


def build_bass():
    nc = bacc.Bacc(None, num_devices=NCORES)

    xT = nc.dram_tensor("xT", [B, C, T], f32, kind="ExternalInput")
    # per-core qkv weights: [3(q,k,v), chunk, row-in-chunk, 2*D]
    w_in = nc.dram_tensor("w", [3, NCC, 128, 2 * D], f32, kind="ExternalInput")
    # proj_w.T chunked: [chunk, row-in-chunk, C]
    pw_in = nc.dram_tensor("pw", [NCC, 128, C], f32, kind="ExternalInput")
    bias_in = nc.dram_tensor("biasb", [128, C], f32, kind="ExternalInput")
    y_out = nc.dram_tensor("y", [B * T // NCORES, C], f32, kind="ExternalOutput")

    ident_np = np.eye(128, dtype=np.float32)
    tri_np = (np.arange(128)[None, :] >= np.arange(128)[:, None]).astype(np.float32)
    ident_dram = nc.inline_tensor(ident_np, name="ident")
    tri_dram = nc.inline_tensor(tri_np, name="trimask")
    onescols_dram = nc.inline_tensor(np.ones((128, NKC), np.float32), name="onescols")

    with tile.TileContext(nc, num_cores=NCORES) as tc:
        with (
            tc.tile_pool(name="dram", bufs=1, space="DRAM") as dpool,
            tc.tile_pool(name="consts", bufs=1) as consts,
            tc.tile_pool(name="xt", bufs=2) as xt_pool,
            tc.tile_pool(name="qt", bufs=2) as qt_pool,
            tc.tile_pool(name="kt", bufs=2) as kt_pool,
            tc.tile_pool(name="vt", bufs=1) as vt_pool,
            tc.tile_pool(name="vaug", bufs=2) as vaug_pool,
            tc.tile_pool(name="e", bufs=3) as e_pool,
            tc.tile_pool(name="onorm", bufs=2) as onorm_pool,
            tc.tile_pool(name="small", bufs=2) as small_pool,
            tc.tile_pool(name="proj", bufs=2) as proj_pool,
            tc.tile_pool(name="ytile", bufs=2) as y_pool,
            tc.tile_pool(name="ps_a", bufs=2, space="PSUM") as ps_a,
            tc.tile_pool(name="ps_s", bufs=2, space="PSUM") as ps_s,
            tc.tile_pool(name="ps_o", bufs=1, space="PSUM") as ps_o,
        ):
            sends = [
                dpool.tile([NCORES, 2 * D, TS], f32, tag="send", name=f"send{b}")
                for b in range(B)
            ]
            recvs = [
                dpool.tile([NCORES, 2 * D, TS], f32, tag="recv", name=f"recv{b}")
                for b in range(B)
            ]

            w_sb = consts.tile([128, 3, NCC, 2 * D], f32r)
            nc.sync.dma_start(
                out=w_sb[:], in_=w_in.rearrange("p i r c -> r p i c").bitcast(f32r)
            )
            ident_sb = consts.tile([128, 128], f32)
            nc.sync.dma_start(out=ident_sb[:], in_=ident_dram[:])
            tri_sb = consts.tile([128, 128], f32r)
            nc.sync.dma_start(out=tri_sb[:], in_=tri_dram[:].bitcast(f32r))
            pw_sb = consts.tile([128, NCC, C], f32r)
            nc.sync.dma_start(
                out=pw_sb[:], in_=pw_in.rearrange("i r e -> r i e").bitcast(f32r)
            )
            bias_sb = consts.tile([128, C], f32)
            nc.sync.dma_start(out=bias_sb[:], in_=bias_in[:])

            def emit_proj(b):
                """Output projection for batch b's tokens (y rows b*256..+256)."""
                oall = proj_pool.tile(
                    [128, NCORES, TS], f32r, tag="oall", name=f"oall{b}"
                )
                nc.sync.dma_start(
                    out=oall[:],
                    in_=recvs[b][:].rearrange("i p t -> p i t").bitcast(f32r),
                )
                for s in range(2):          # token sub-tiles of 128
                    for n in range(2):      # out-embd halves of 512
                        pso = ps_a.tile(
                            [128, 512], f32, tag="qkv", name=f"pso{b}_{s}_{n}"
                        )
                        for i in range(NCC):
                            nc.tensor.matmul(
                                pso[:],
                                lhsT=oall[:, i, s * 128:(s + 1) * 128],
                                rhs=pw_sb[:, i, n * 512:(n + 1) * 512],
                                start=(i == 0),
                                stop=(i == NCC - 1),
                            )
                        yt = y_pool.tile(
                            [128, 512], f32, tag="yt", name=f"yt{b}_{s}_{n}"
                        )
                        nc.vector.tensor_add(
                            yt[:], pso[:], bias_sb[:, n * 512:(n + 1) * 512]
                        )
                        nc.sync.dma_start(
                            out=y_out[
                                b * TS + s * 128:b * TS + (s + 1) * 128,
                                n * 512:(n + 1) * 512,
                            ],
                            in_=yt[:],
                        )

            def make_qkv(b):
                """Emit xt DMAs eagerly; return (tiles, PE work units) for batch b.

                Each unit is ~1.8us of PE work with no ACT dependency; they are
                interleaved into the previous batch's attention rounds to keep
                the PE dense (and therefore at the warm 2.4 GHz clock) while
                the ACT engine works through the Exp stream."""
                QT = qt_pool.tile([128, T], f32r, tag="QT", name=f"QT{b}")
                KT = kt_pool.tile([128, T], f32r, tag="KT", name=f"KT{b}")
                VT = vt_pool.tile([128, T], f32, tag="VT", name=f"VT{b}")
                VA = vaug_pool.tile([128, NKC, 130], f32r, tag="VA", name=f"VA{b}")
                nc.sync.dma_start(out=VA[:, :, 64], in_=onescols_dram[:].bitcast(f32r))
                nc.sync.dma_start(out=VA[:, :, 129], in_=onescols_dram[:].bitcast(f32r))
                dests = [QT, KT, VT]
                xts = []
                for t4 in range(NJ):
                    xt = xt_pool.tile([128, NCC, TQ], f32r, tag="xt",
                                      name=f"xt{b}_{t4}")
                    nc.sync.dma_start(
                        out=xt[:],
                        in_=xT[b, :, t4 * TQ:(t4 + 1) * TQ]
                        .rearrange("(i p) t -> p i t", p=128)
                        .bitcast(f32r),
                    )
                    xts.append(xt)
                units = []

                def chain(t4, p3):
                    ps = ps_a.tile([128, TQ], f32, tag="qkv",
                                   name=f"qkv{b}_{t4}_{p3}")
                    for i in range(NCC):
                        nc.tensor.matmul(
                            ps[:],
                            lhsT=w_sb[:, p3, i, :],
                            rhs=xts[t4][:, i, :],
                            start=(i == 0),
                            stop=(i == NCC - 1),
                        )
                    nc.vector.tensor_copy(
                        dests[p3][:, t4 * TQ:(t4 + 1) * TQ], ps[:]
                    )

                def transp(kc):
                    pst = ps_a.tile([128, 128], f32, tag="qkv", name=f"pst{b}_{kc}")
                    nc.tensor.transpose(
                        pst[:], VT[:, kc * 128:(kc + 1) * 128], ident_sb[:]
                    )
                    out_ap = VA[:, kc, :].rearrange("p (g s) -> p g s", s=65)[
                        :, :, 0:64
                    ]
                    in_ap = pst[:].rearrange("p (g s) -> p g s", s=64)
                    nc.vector.tensor_copy(out_ap, in_ap)

                for t4 in range(NJ):
                    for p3 in range(3):
                        units.append(lambda t4=t4, p3=p3: chain(t4, p3))
                for kc2 in range(NKC // 2):
                    units.append(
                        lambda kc2=kc2: (transp(2 * kc2), transp(2 * kc2 + 1))
                    )
                return (QT, KT, VA), units

            cur, units = make_qkv(0)
            for u in units:
                u()

            for b in range(B):
                QT, KT, VA = cur
                if b + 1 < B:
                    cur, units = make_qkv(b + 1)
                else:
                    units = []
                ui = 0

                # attention for this batch, both heads, chunk-PAIR pipelined
                for j in range(NJ):
                    po = [
                        ps_o.tile([65, TQ], f32, tag=f"o{h}", name=f"po{h}_{b}_{j}")
                        for h in (0, 1)
                    ]
                    npairs = 2 * (j + 1)
                    av_queue = []  # exp'd pairs whose A'V is pending

                    def emit_av(item, j=j, po=po, VA=VA):
                        E2, cpair = item
                        for h in (0, 1):
                            for ci in (0, 1):
                                c = 2 * cpair + ci
                                m = c - 4 * j
                                cs = (
                                    slice(ci * TQ, (ci + 1) * TQ)
                                    if m < 0
                                    else slice(ci * TQ + m * 128, (ci + 1) * TQ)
                                )
                                ocs = slice(0, TQ) if m < 0 else slice(m * 128, TQ)
                                nc.tensor.matmul(
                                    po[h][:, ocs],
                                    lhsT=VA[:, c, 65 * h:65 * h + 65],
                                    rhs=E2[h][:, cs],
                                    start=(c == 0),
                                    stop=(c == 4 * j + 3),
                                )

                    for cpair in range(npairs):
                        pss2 = [
                            ps_s.tile([128, 2 * TQ], f32, tag="pss",
                                      name=f"pss{b}_{j}_{cpair}_{h}")
                            for h in (0, 1)
                        ]
                        # h-inner order alternates PE row groups -> the two
                        # heads' K=64 score matmuls run concurrently
                        for ci in (0, 1):
                            c = 2 * cpair + ci
                            for h in (0, 1):
                                nc.tensor.matmul(
                                    pss2[h][:, ci * TQ:(ci + 1) * TQ],
                                    lhsT=KT[
                                        64 * h:64 * (h + 1), c * 128:(c + 1) * 128
                                    ],
                                    rhs=QT[64 * h:64 * (h + 1), j * TQ:(j + 1) * TQ],
                                    start=True,
                                    stop=True,
                                    tile_position=(64 * h, 0),
                                )
                        E2 = []
                        for h in (0, 1):
                            E = e_pool.tile(
                                [128, 2 * TQ], f32r, tag="E",
                                name=f"E{b}_{j}_{cpair}_{h}",
                            )
                            nc.scalar.activation(E[:], pss2[h][:], AF.Exp, scale=0.125)
                            # triangular mask on diagonal chunks
                            for ci in (0, 1):
                                c = 2 * cpair + ci
                                m = c - 4 * j
                                if m >= 0:
                                    sl = slice(
                                        ci * TQ + m * 128, ci * TQ + (m + 1) * 128
                                    )
                                    nc.vector.tensor_mul(E[:, sl], E[:, sl], tri_sb[:])
                            E2.append(E)
                        av_queue.append((E2, cpair))
                        if len(av_queue) > 1:
                            emit_av(av_queue.pop(0))
                        if ui < len(units):
                            units[ui]()
                            ui += 1
                    emit_av(av_queue.pop(0))

                    # copy O.T out of PSUM (frees accumulator), then normalize
                    for h in (0, 1):
                        on_raw = onorm_pool.tile(
                            [65, TQ], f32, tag="onr", name=f"onr{b}_{j}_{h}"
                        )
                        nc.vector.tensor_copy(on_raw[:], po[h][:])
                        rec = small_pool.tile([1, TQ], f32, tag="rec")
                        nc.vector.reciprocal(rec[:], on_raw[64:65, :])
                        den = small_pool.tile([64, TQ], f32, tag="den")
                        nc.gpsimd.partition_broadcast(den[:], rec[:])
                        on = onorm_pool.tile(
                            [64, TQ], f32, tag="on", name=f"on{b}_{j}_{h}"
                        )
                        nc.vector.tensor_mul(on[:], on_raw[0:64, :], den[:])
                        for half in (0, 1):
                            nc.sync.dma_start(
                                out=sends[b][2 * j + half, 64 * h:64 * (h + 1), :],
                                in_=on[:, half * TS:(half + 1) * TS],
                            )
                for u in units[ui:]:
                    u()
                nc.gpsimd.collective_compute(
                    "AllToAll",
                    mybir.AluOpType.bypass,
                    replica_groups=[list(range(NCORES))],
                    ins=[sends[b][:].opt()],
                    outs=[recvs[b][:].opt()],
                )
                if b >= 1:
                    emit_proj(b - 1)
            emit_proj(B - 1)
    nc.finalize()
    return nc


_NC_CACHE = {}


def _get_nc():
    if "nc" not in _NC_CACHE:
        _NC_CACHE["nc"] = build_bass()
    return _NC_CACHE["nc"]


def _prep_inputs(x, Wk, Wq, Wv, proj_w, proj_b):
    x = np.ascontiguousarray(np.asarray(x, dtype=np.float32))
    xT = np.ascontiguousarray(x.transpose(0, 2, 1))  # [B, C, T]
    pw_r = np.ascontiguousarray(np.asarray(proj_w, np.float32).T).reshape(NCC, 128, C)
    biasb = np.ascontiguousarray(
        np.broadcast_to(np.asarray(proj_b, np.float32), (128, C))
    )
    in_maps = []
    for core in range(NCORES):
        h0 = 2 * core

        def pack(W):
            W2 = np.concatenate(
                [np.asarray(W[h0], np.float32), np.asarray(W[h0 + 1], np.float32)],
                axis=1,
            )  # [C, 2D]
            return W2.reshape(NCC, 128, 2 * D)

        wq = np.stack([pack(Wq), pack(Wk), pack(Wv)], axis=0)  # [3, NCC, 128, 2D]
        in_maps.append(
            {
                "xT": xT,
                "w": np.ascontiguousarray(wq),
                "pw": pw_r,
                "biasb": biasb,
            }
        )
    return in_maps


def _assemble(results):
    """Core r's y rows [b*256, (b+1)*256) = tokens
    b*2048 + (r//2)*512 + (r%2)*256 + [0, 256)."""
    out = np.empty((B * T, C), np.float32)
    for r in range(NCORES):
        y = results[r]["y"]
        base = (r // 2) * TQ + (r % 2) * TS
        for b in range(B):
            out[b * T + base:b * T + base + TS] = y[b * TS:(b + 1) * TS]
    return out.reshape(B, T, C)


def kernel(x, Wk, Wq, Wv, proj_w, proj_b, _trace=False, _trace_kwargs=None):
    in_maps = _prep_inputs(x, Wk, Wq, Wv, proj_w, proj_b)
    nc = _get_nc()
    kw = {}
    if _trace:
        kw = dict(trace=True, trace_kwargs=_trace_kwargs or {})
    res = run_bass_kernel_spmd(nc, in_maps, core_ids=list(range(NCORES)), **kw)
    out = _assemble(res.results)
    if _trace:
        return out, res
    return out


if __name__ == "__main__":
    d = np.load("/root/problem/cache_io.npz")
    out = kernel(d["x"], d["Wk"], d["Wq"], d["Wv"], d["proj_w"], d["proj_b"])
    ref = d["ref"]
    err = np.abs(out - ref).max() / np.abs(ref).max()
    print("Relative error:", err)
